# revision 1
# baseline (speedup 1.0000x reference)
"""Trainium2 Bass kernel for nn_Attention_Conv_surface (gnn_message_passing).

Math (per batch b):
  neighbors = vertices[idx]                          # (V, N, 3)
  dirn = normalize(neighbors - vertices[:, None])    # (V, N, 3)
  theta_d = sum_s max_n relu(dirn @ sdn_d)           # (V, K) for d in {q,k,v}
  qkv = theta @ W.T + b ; MHA over full VxV ; out = attn_out @ Wo.T + bo

Device strategy:
  * max_n relu(x) == relu(max_n x); normalize scale folded into dirn.
  * The PE rounds matmul operands to ~bf16, so every precision-critical matmul
    runs as a bf16 hi/lo-split product.  The cross terms are folded into ONE
    matmul by stacking hi/lo blocks along the contraction dim (contraction
    length is free on the PE).
  * theta: dirn tiles are split into (dirh, dirh, dirl) 9-row groups per
    neighbor, PE-transposed to T4 [126, 512]; a host-built sparse lhsT per
    (chunk, n) carries (sdh, sdl, sdh) at the matching rows, so one matmul per
    (chunk, vgroup, n) yields the full bf16x3 product.  Max over n is a DVE
    tensor-tensor chain over PSUM tiles; relu after; the support-sum uses DVE
    partition-pair adds (exact fp32).
  * attention: scores are computed transposed with augmented operands
    qa=[qh/4;-m], ka=[kh;1] in an x3 block layout (blocks at partitions
    0/32/64); m comes from a cheap single-bf16 max pass (any shift works).
    exp on ACT writes bf16 e; PV augments v with a ones-row so the softmax
    denominator falls out of the same matmul; fp32 PE transposes are exact.

Sharding: 8 cores = (batch 0..3) x (query half 0..1). Each core computes
k/v thetas for the full batch (duplicated within the pair) and q theta +
attention for its own 1024 queries. Identical SPMD program; the query half is
selected by feeding each core a half-rolled permutation of its batch's data.
"""

import numpy as np

BS, V, N, S, K, H = 4, 2048, 32, 4, 64, 4
DK = K // H
VQ = V // 2          # queries per core
NVT = V // 128       # vertex tiles per batch (16)
NCH = 6              # sk chunks of 128 (768 total = 3 dirs * 256)
EPS = 1e-12
NGRP = [(0, 14), (14, 14), (28, 4)]   # (n0, size) neighbor groups per T4 tile
RROWS = [126, 126, 36]
AUG = 81             # rows used of the x3-block score operands

_CACHE = {}


def _grp(n):
    t = 0 if n < 14 else (1 if n < 28 else 2)
    return t, n - NGRP[t][0]


def _build_program():
    import concourse.bass as bass
    import concourse.mybir as mybir
    import concourse.tile as tile
    from concourse import bacc
    from contextlib import ExitStack

    f32 = mybir.dt.float32
    bf16 = mybir.dt.bfloat16
    i32 = mybir.dt.int32
    Alu = mybir.AluOpType
    Act = mybir.ActivationFunctionType

    nc = bacc.Bacc("TRN2", target_bir_lowering=False, debug=False)

    # ---- DRAM I/O ----
    verts_d = nc.dram_tensor("verts", [V, 3], f32, kind="ExternalInput").ap()
    gath_d = nc.dram_tensor("gath", [V, N, 3], f32, kind="ExternalInput").ap()
    sdnN_d = nc.dram_tensor("sdnN", [NCH, N, 126, 128], bf16, kind="ExternalInput").ap()
    ident_d = nc.dram_tensor("ident", [128, 128], f32, kind="ExternalInput").ap()
    identb_d = nc.dram_tensor("identb", [128, 128], bf16, kind="ExternalInput").ap()
    wst_d = nc.dram_tensor("wst", [4, 128, K], bf16, kind="ExternalInput").ap()
    wl_d = nc.dram_tensor("wl", [4, K, K], bf16, kind="ExternalInput").ap()
    bh_d = nc.dram_tensor("bh", [DK, 16], f32, kind="ExternalInput").ap()
    bo_d = nc.dram_tensor("bo_col", [K, 1], f32, kind="ExternalInput").ap()
    ones_row_d = nc.dram_tensor("ones_row", [1, V], bf16, kind="ExternalInput").ap()
    ones_col_d = nc.dram_tensor("ones_col", [128, V // 128], bf16, kind="ExternalInput").ap()
    out_d = nc.dram_tensor("out_t", [K, VQ], f32, kind="ExternalOutput").ap()

    with tile.TileContext(nc) as tc:
        with (
            tc.tile_pool(name="const", bufs=1) as cpool,
            tc.tile_pool(name="ps", bufs=4, space="PSUM") as pspool,
            tc.tile_pool(name="pst", bufs=2, space="PSUM") as pstpool,
            tc.tile_pool(name="psx", bufs=2, space="PSUM") as psxpool,
        ):
            # ---- persistent constants ----
            ident = cpool.tile([128, 128], f32)
            nc.sync.dma_start(ident[:], ident_d[:])
            identb = cpool.tile([128, 128], bf16)
            nc.sync.dma_start(identb[:], identb_d[:])
            wst = cpool.tile([128, 4, K], bf16)
            nc.sync.dma_start(wst[:], wst_d.rearrange("w a b -> a w b"))
            wl = cpool.tile([K, 4, K], bf16)
            nc.sync.dma_start(wl[:], wl_d.rearrange("w a b -> a w b"))
            bh = cpool.tile([DK, 16], f32)
            nc.sync.dma_start(bh[:], bh_d[:])
            bo = cpool.tile([K, 1], f32)
            nc.sync.dma_start(bo[:], bo_d[:])
            # persistent theta^T splits [h-rows 0:64 | l-rows 64:128]
            th_q = cpool.tile([128, VQ], bf16)
            th_k = cpool.tile([128, V], bf16)
            th_v = cpool.tile([128, V], bf16)
            # score operand tiles (x3 block layout), zeroed once
            qa3 = cpool.tile([96, VQ], bf16)
            nc.vector.memset(qa3[:], 0.0)
            ka3 = cpool.tile([96, V], bf16)
            nc.vector.memset(ka3[:], 0.0)
            nc.sync.dma_start(ka3[DK : DK + 1, :], ones_row_d[:])
            nc.sync.dma_start(ka3[64 + DK : 64 + DK + 1, :], ones_row_d[:])
            va = cpool.tile([128, V // 128, DK + 1], bf16)
            nc.sync.dma_start(
                va[:, :, DK : DK + 1].rearrange("p a b -> p (a b)"), ones_col_d[:]
            )
            O = cpool.tile([128, 8, K], f32)       # [128q, qt, 64]
            OT2 = cpool.tile([128, VQ], bf16)      # [OTh | OTl]
            outsb = cpool.tile([K, VQ], f32)

            theta_stack = ExitStack()
            vtpool = theta_stack.enter_context(tc.tile_pool(name="vt", bufs=3))
            lhspool = theta_stack.enter_context(tc.tile_pool(name="lhs", bufs=2))
            accpool = theta_stack.enter_context(tc.tile_pool(name="acc", bufs=3))
            t4pool = theta_stack.enter_context(tc.tile_pool(name="t4p", bufs=1))
            xpool = theta_stack.enter_context(tc.tile_pool(name="xp", bufs=1))

            # ---- phase 1: per-vtile edge math + split + transposes -> T4 ----
            t4s = [[None] * 3 for _ in range(4)]
            for g in range(4):
                for t in range(3):
                    t4_t = t4pool.tile([RROWS[t], 512], bf16, tag=f"t4_{g}_{t}")
                    t4s[g][t] = t4_t
            for vt in range(NVT):
                g, vt4 = vt // 4, vt % 4
                vsl = slice(vt * 128, vt * 128 + 128)
                gath = vtpool.tile([128, N, 3], f32, tag="gath")
                nc.sync.dma_start(gath[:], gath_d[vsl, :, :])
                cent = vtpool.tile([128, 3], f32, tag="cent")
                nc.sync.dma_start(cent[:], verts_d[vsl, :])
                diff = vtpool.tile([128, N, 3], f32, tag="diff")
                for c in range(3):
                    nc.vector.tensor_tensor(
                        out=diff[:, :, c],
                        in0=gath[:, :, c],
                        in1=cent[:, c : c + 1].to_broadcast([128, N]),
                        op=Alu.subtract,
                    )
                dsq = vtpool.tile([128, N, 3], f32, tag="dsq")
                nc.scalar.square(dsq[:], diff[:])
                nsq = vtpool.tile([128, N], f32, tag="nsq")
                nc.vector.reduce_sum(nsq[:], dsq[:], axis=mybir.AxisListType.X)
                nrm = vtpool.tile([128, N], f32, tag="nrm")
                nc.scalar.sqrt(nrm[:], nsq[:])
                nc.vector.tensor_scalar_max(nrm[:], nrm[:], EPS)
                invn = vtpool.tile([128, N], f32, tag="invn")
                nc.vector.reciprocal(invn[:], nrm[:])
                dirn = vtpool.tile([128, N, 3], f32, tag="dirn")
                nc.vector.tensor_tensor(
                    out=dirn[:],
                    in0=diff[:],
                    in1=invn[:].to_broadcast([128, N, 3]),
                    op=Alu.mult,
                )
                dirh = vtpool.tile([128, N, 3], bf16, tag="dirh")
                nc.vector.tensor_copy(dirh[:], dirn[:])
                dirl = vtpool.tile([128, N, 3], bf16, tag="dirl")
                nc.vector.tensor_tensor(
                    out=dirl[:], in0=dirn[:], in1=dirh[:], op=Alu.subtract
                )
                for t, (n0, gsz) in enumerate(NGRP):
                    dx = vtpool.tile([128, gsz, 9], bf16, tag=f"dx{t}")
                    nc.vector.tensor_copy(dx[:, :, 0:3], dirh[:, n0 : n0 + gsz, :])
                    nc.vector.tensor_copy(dx[:, :, 3:6], dirh[:, n0 : n0 + gsz, :])
                    nc.vector.tensor_copy(dx[:, :, 6:9], dirl[:, n0 : n0 + gsz, :])
                    tp = pstpool.tile([126, 128], bf16, tag="small")
                    nc.tensor.transpose(
                        tp[0 : 9 * gsz, :],
                        dx[:].rearrange("p a b -> p (a b)"),
                        identb[:],
                    )
                    nc.scalar.copy(
                        t4s[g][t][:, vt4 * 128 : vt4 * 128 + 128],
                        tp[0 : RROWS[t], :],
                    )

            # ---- phase 2: theta matmuls; TT-chain max over n; s-sum on DVE ----
            xq = xpool.tile([K, VQ], f32, tag="xq")
            xk = xpool.tile([K, V], f32, tag="xk")
            xv = xpool.tile([K, V], f32, tag="xv")
            xdst = {0: xq, 1: xk, 2: xv}

            for pr in range(3):
                lhsA = lhspool.tile([126, N, 128], bf16, tag="lhsA")
                nc.sync.dma_start(
                    lhsA[:], sdnN_d[2 * pr, :, :, :].rearrange("n p m -> p n m")
                )
                lhsB = lhspool.tile([126, N, 128], bf16, tag="lhsB")
                nc.sync.dma_start(
                    lhsB[:], sdnN_d[2 * pr + 1, :, :, :].rearrange("n p m -> p n m")
                )
                ngr = 2 if pr == 0 else 4  # q chunks: own half only
                for g in range(ngr):
                    parts = []
                    for ch_i, lhs in ((0, lhsA), (1, lhsB)):
                        acc = accpool.tile([128, 512], f32, tag="acc")
                        for n in range(N):
                            t, j = _grp(n)
                            R = RROWS[t]
                            ps = pspool.tile([128, 512], f32, tag="big")
                            nc.tensor.matmul(
                                out=ps[:],
                                lhsT=lhs[0:R, n, :],
                                rhs=t4s[g][t][:],
                                start=True,
                                stop=True,
                            )
                            if n == 0:
                                nc.scalar.copy(acc[:], ps[:])
                            else:
                                nc.vector.tensor_tensor(
                                    out=acc[:], in0=ps[:], in1=acc[:], op=Alu.max
                                )
                        rlo = accpool.tile([K, 512], f32, tag="rlo")
                        nc.scalar.activation(rlo[:], acc[0:K, :], Act.Relu)
                        rhi = accpool.tile([K, 512], f32, tag="rhi")
                        nc.scalar.activation(rhi[:], acc[K:128, :], Act.Relu)
                        part = accpool.tile([K, 512], f32, tag=f"part{ch_i}")
                        nc.vector.tensor_tensor(
                            out=part[:], in0=rlo[:], in1=rhi[:], op=Alu.add,
                        )
                        parts.append(part)
                    nc.vector.tensor_tensor(
                        out=xdst[pr][:, g * 512 : g * 512 + 512],
                        in0=parts[0][:],
                        in1=parts[1][:],
                        op=Alu.add,
                    )

            # theta hi/lo splits [128, V]: rows 0:64 hi, 64:128 lo
            for xsb, th in ((xq, th_q), (xk, th_k), (xv, th_v)):
                nc.vector.tensor_copy(th[0:K, :], xsb[:])
                nc.vector.tensor_tensor(
                    out=th[K:128, :], in0=xsb[:], in1=th[0:K, :], op=Alu.subtract
                )
            theta_stack.close()

            # ---- phase 3+4: per-head projection + attention ----
            attn_stack = ExitStack()
            atpool = attn_stack.enter_context(tc.tile_pool(name="attn", bufs=2))
            epool = attn_stack.enter_context(tc.tile_pool(name="epool", bufs=3))

            for h in range(H):
                hsl = slice(DK * h, DK * h + DK)
                # projections for this head: 2-matmul hi/lo scheme
                heads = {}
                for wi, (th, vv, nm) in enumerate(
                    ((th_q, VQ, "qf"), (th_k, V, "kf"), (th_v, V, "vf"))
                ):
                    hf = atpool.tile([DK, vv], f32, tag=nm)
                    heads[nm] = hf
                    for tt in range(vv // 512):
                        sl = slice(tt * 512, tt * 512 + 512)
                        pp = psxpool.tile([DK, 512], f32, tag="xps")
                        nc.tensor.matmul(
                            out=pp[:], lhsT=wst[:, wi, hsl], rhs=th[:, sl],
                            start=True, stop=False,
                        )
                        nc.tensor.matmul(
                            out=pp[:], lhsT=wl[:, wi, hsl], rhs=th[0:K, sl],
                            start=False, stop=True,
                        )
                        nc.scalar.activation(
                            hf[:, sl], pp[:], Act.Identity,
                            bias=bh[:, wi * 4 + h : wi * 4 + h + 1],
                        )
                qf, kf, vf = heads["qf"], heads["kf"], heads["vf"]

                # ka3 blocks: [0:16]=kah, [32:48]=kal, [64:80]=kah
                nc.vector.tensor_copy(ka3[0:DK, :], kf[:])
                nc.vector.tensor_tensor(
                    out=ka3[32 : 32 + DK, :], in0=kf[:], in1=ka3[0:DK, :],
                    op=Alu.subtract,
                )
                nc.vector.tensor_copy(ka3[64 : 64 + DK, :], ka3[0:DK, :])
                # qa3 blocks: [0:16]=qah, [32:48]=qah, [64:80]=qal (q/4)
                q4 = atpool.tile([DK, VQ], f32, tag="q4")
                nc.scalar.mul(q4[:], qf[:], 0.25)
                nc.vector.tensor_copy(qa3[0:DK, :], q4[:])
                nc.vector.tensor_copy(qa3[32 : 32 + DK, :], qa3[0:DK, :])
                nc.vector.tensor_tensor(
                    out=qa3[64 : 64 + DK, :], in0=q4[:], in1=qa3[0:DK, :],
                    op=Alu.subtract,
                )
                # va: v head transposed (exact fp32), cast bf16
                for kt in range(V // 128):
                    vps = pstpool.tile([128, DK], f32, tag="small")
                    nc.tensor.transpose(
                        vps[:], vf[:, kt * 128 : kt * 128 + 128], ident[0:DK, 0:DK]
                    )
                    nc.scalar.copy(va[:, kt, 0:DK], vps[:])

                # m-pass on hi blocks (coarse max; any shift is valid)
                mcols = atpool.tile([128, 8], f32, tag="mcols")
                for qt in range(8):
                    m4 = atpool.tile([128, 4], f32, tag="m4")
                    for k4 in range(4):
                        sps = pspool.tile([128, 512], f32, tag="big")
                        nc.tensor.matmul(
                            out=sps[:],
                            lhsT=qa3[0:DK, qt * 128 : qt * 128 + 128],
                            rhs=ka3[0:DK, k4 * 512 : k4 * 512 + 512],
                            start=True,
                            stop=True,
                        )
                        nc.vector.reduce_max(
                            m4[:, k4 : k4 + 1], sps[:], axis=mybir.AxisListType.X
                        )
                    nc.vector.tensor_reduce(
                        out=mcols[:, qt : qt + 1], in_=m4[:],
                        axis=mybir.AxisListType.X, op=Alu.max,
                    )
                nc.vector.tensor_scalar_mul(mcols[:], mcols[:], -1.0)
                mrow_ps = pstpool.tile([8, 128], f32, tag="small")
                nc.tensor.transpose(mrow_ps[:], mcols[:], ident[:])
                msb = atpool.tile([8, 128], bf16, tag="msb")
                nc.scalar.copy(msb[:], mrow_ps[:])
                for qt in range(8):
                    nc.sync.dma_start(
                        qa3[DK : DK + 1, qt * 128 : qt * 128 + 128],
                        msb[qt : qt + 1, :],
                    )

                # ST' + exp + PV
                for qs in range(VQ // 512):
                    pv = psxpool.tile([DK + 1, 512], f32, tag="xps")
                    for kt in range(V // 128):
                        stp = pspool.tile([128, 512], f32, tag="big")
                        nc.tensor.matmul(
                            out=stp[:],
                            lhsT=ka3[0:AUG, kt * 128 : kt * 128 + 128],
                            rhs=qa3[0:AUG, qs * 512 : qs * 512 + 512],
                            start=True,
                            stop=True,
                        )
                        e = epool.tile([128, 512], bf16, tag="e")
                        nc.scalar.activation(e[:], stp[:], Act.Exp)
                        nc.tensor.matmul(
                            out=pv[:],
                            lhsT=va[:, kt, :],
                            rhs=e[:],
                            start=(kt == 0),
                            stop=(kt == V // 128 - 1),
                        )
                    pvs = atpool.tile([DK + 1, 512], f32, tag="pvs")
                    nc.scalar.copy(pvs[:], pv[:])
                    for q4i in range(4):
                        qt = qs * 4 + q4i
                        pq = pstpool.tile([128, DK + 1], f32, tag="small")
                        nc.tensor.transpose(
                            pq[:], pvs[:, q4i * 128 : q4i * 128 + 128],
                            ident[0 : DK + 1, 0 : DK + 1],
                        )
                        rz = atpool.tile([128, 1], f32, tag="rz")
                        nc.vector.reciprocal(rz[:], pq[:, DK : DK + 1])
                        nc.vector.tensor_scalar_mul(O[:, qt, hsl], pq[:, 0:DK], rz[:])

            # ---- phase 5: O hi/lo transpose + final projection ----
            for qt in range(8):
                qsl = slice(qt * 128, qt * 128 + 128)
                oh = atpool.tile([128, K], bf16, tag="oh")
                nc.vector.tensor_copy(oh[:], O[:, qt, :])
                ol = atpool.tile([128, K], bf16, tag="ol")
                nc.vector.tensor_tensor(
                    out=ol[:], in0=O[:, qt, :], in1=oh[:], op=Alu.subtract
                )
                oph = pstpool.tile([K, 128], bf16, tag="small")
                nc.tensor.transpose(oph[:], oh[:], identb[:])
                nc.scalar.copy(OT2[0:K, qsl], oph[:])
                opl = pstpool.tile([K, 128], bf16, tag="small")
                nc.tensor.transpose(opl[:], ol[:], identb[:])
                nc.scalar.copy(OT2[K:128, qsl], opl[:])
            for qs in range(VQ // 512):
                sl = slice(qs * 512, qs * 512 + 512)
                fp = psxpool.tile([K, 512], f32, tag="xps")
                nc.tensor.matmul(
                    out=fp[:], lhsT=wst[:, 3, :], rhs=OT2[:, sl],
                    start=True, stop=False,
                )
                nc.tensor.matmul(
                    out=fp[:], lhsT=wl[:, 3, :], rhs=OT2[0:K, sl],
                    start=False, stop=True,
                )
                nc.scalar.activation(outsb[:, sl], fp[:], Act.Identity, bias=bo[:])
            nc.sync.dma_start(out_d[:], outsb[:])
            attn_stack.close()

    nc.compile()
    return nc


def _host_prep(inputs):
    """Build the 8 per-core input maps from full inputs."""
    import ml_dtypes

    bfd = ml_dtypes.bfloat16
    verts = np.ascontiguousarray(np.asarray(inputs["vertices"], dtype=np.float32))
    idx = np.ascontiguousarray(np.asarray(inputs["neighbor_index"]).astype(np.int32))

    sd = np.concatenate(
        [np.asarray(inputs["q_dirs"]), np.asarray(inputs["k_dirs"]),
         np.asarray(inputs["v_dirs"])], axis=1
    ).astype(np.float32)  # [3, 768]
    nrm = np.sqrt((sd * sd).sum(0, dtype=np.float32), dtype=np.float32)
    sdn = (sd / np.maximum(nrm, np.float32(EPS))).astype(np.float32)
    sdh = sdn.astype(bfd)
    sdl = (sdn - sdh.astype(np.float32)).astype(bfd)

    # sparse lhsT bank: [ch, n, 126, 128]; rows 9j+{0..2}=sdh, {3..5}=sdl,
    # {6..8}=sdh at this chunk's 128 columns
    sdnN = np.zeros((NCH, N, 126, 128), bfd)
    for ch in range(NCH):
        bh_ = sdh[:, ch * 128 : ch * 128 + 128]
        bl_ = sdl[:, ch * 128 : ch * 128 + 128]
        for n in range(N):
            t, j = _grp(n)
            sdnN[ch, n, 9 * j : 9 * j + 3, :] = bh_
            sdnN[ch, n, 9 * j + 3 : 9 * j + 6, :] = bl_
            sdnN[ch, n, 9 * j + 6 : 9 * j + 9, :] = bh_

    # weights: wst [4, 128, 64] = [Wh.T ; Wh.T], wl [4, 64, 64] = Wl.T
    wst = np.zeros((4, 128, K), bfd)
    wlo = np.zeros((4, K, K), bfd)
    for wi, kk in enumerate(("Wq", "Wk", "Wv", "Wo")):
        wt_ = np.asarray(inputs[kk], dtype=np.float32).T
        wh_ = wt_.astype(bfd)
        wst[wi, 0:K, :] = wh_
        wst[wi, K:128, :] = wh_
        wlo[wi] = (wt_ - wh_.astype(np.float32)).astype(bfd)

    bh = np.zeros((DK, 16), np.float32)
    for wi, kk in enumerate(("bq", "bk", "bv", "bo")):
        bb_ = np.asarray(inputs[kk], dtype=np.float32)
        for h in range(H):
            bh[:, wi * 4 + h] = bb_[DK * h : DK * h + DK]
    bo_col = np.asarray(inputs["bo"], dtype=np.float32).reshape(K, 1)

    common = {
        "sdnN": sdnN,
        "ident": np.eye(128, dtype=np.float32),
        "identb": np.eye(128, dtype=np.float32).astype(bfd),
        "wst": wst,
        "wl": wlo,
        "bh": bh,
        "bo_col": bo_col,
        "ones_row": np.ones((1, V), bfd),
        "ones_col": np.ones((128, V // 128), bfd),
    }

    in_maps = []
    for core in range(8):
        bb, half = core // 2, core % 2
        if half == 0:
            vb, ib = verts[bb], idx[bb]
        else:
            perm = np.concatenate([np.arange(VQ, V), np.arange(0, VQ)])
            vb = verts[bb][perm]
            ib = np.where(idx[bb][perm] >= VQ, idx[bb][perm] - VQ, idx[bb][perm] + VQ)
        in_maps.append({
            "verts": np.ascontiguousarray(vb),
            "gath": np.ascontiguousarray(vb[ib]),
            **common,
        })
    return in_maps


def run(inputs, trace=False, trace_kwargs=None):
    from concourse.bass_utils import run_bass_kernel_spmd

    if "nc" not in _CACHE:
        _CACHE["nc"] = _build_program()
    nc = _CACHE["nc"]
    in_maps = _host_prep(inputs)
    res = run_bass_kernel_spmd(
        nc, in_maps, core_ids=list(range(8)), trace=trace,
        **(trace_kwargs or {}),
    )
    out = np.zeros((BS, V, K), np.float32)
    for core in range(8):
        bb, half = core // 2, core % 2
        ot = res.results[core]["out_t"]  # [64, 1024]
        out[bb, half * VQ : half * VQ + VQ, :] = ot.T
    return out, res


def kernel(**inputs) -> np.ndarray:
    out, _ = run(inputs, trace=False)
    return out


def time_exec(inputs, iters=20):
    """Wall-time the compiled NEFF with device-resident inputs (upload excluded).

    Returns (sec_per_call, out) — an upper bound on per-launch HW exec time
    (includes per-call dispatch through the PJRT/axon path).
    """
    import time
    import jax
    import jax.numpy as jnp
    from jax.sharding import Mesh, PartitionSpec
    from jax.experimental.shard_map import shard_map
    import concourse.mybir as mybir
    from concourse import bass2jax

    if "nc" not in _CACHE:
        _CACHE["nc"] = _build_program()
    nc = _CACHE["nc"]
    in_maps = _host_prep(inputs)
    bass2jax.install_neuronx_cc_hook()

    n_cores = 8
    partition_name = nc.partition_id_tensor.name if nc.partition_id_tensor else None
    in_names, out_names, out_avals = [], [], []
    for alloc in nc.m.functions[0].allocations:
        if not isinstance(alloc, mybir.MemoryLocationSet):
            continue
        name = alloc.memorylocations[0].name
        if alloc.kind == "ExternalInput":
            if name != partition_name:
                in_names.append(name)
        elif alloc.kind == "ExternalOutput":
            out_names.append(name)
            out_avals.append(
                jax.core.ShapedArray(tuple(alloc.tensor_shape),
                                     mybir.dt.np(alloc.dtype))
            )
    n_params = len(in_names)
    all_names = list(in_names) + list(out_names)
    if partition_name is not None:
        all_names.append(partition_name)

    def _body(*args):
        operands = list(args)
        if partition_name is not None:
            operands.append(bass2jax.partition_id_tensor())
        return tuple(bass2jax._bass_exec_p.bind(
            *operands,
            out_avals=tuple(out_avals),
            in_names=tuple(all_names),
            out_names=tuple(out_names),
            lowering_input_output_aliases=(),
            sim_require_finite=True,
            sim_require_nnan=True,
            nc=nc,
        ))

    devices = jax.devices()[:n_cores]
    mesh = Mesh(np.asarray(devices), ("core",))
    n_outs = len(out_names)
    sharded = jax.jit(shard_map(
        _body, mesh=mesh,
        in_specs=(PartitionSpec("core"),) * (n_params + n_outs),
        out_specs=(PartitionSpec("core"),) * n_outs,
        check_rep=False,
    ), keep_unused=True)
    concat_in = [
        jnp.asarray(np.concatenate([np.asarray(in_maps[c][nm])[None] for c in range(n_cores)], 0)
                    .reshape(-1, *np.asarray(in_maps[0][nm]).shape[1:]))
        for nm in in_names
    ]
    concat_zeros = [
        jnp.zeros((n_cores * a.shape[0], *a.shape[1:]), a.dtype) for a in out_avals
    ]
    concat_in = [jax.device_put(x) for x in concat_in]
    out = sharded(*concat_in, *concat_zeros)
    jax.block_until_ready(out)
    t0 = time.time()
    for _ in range(iters):
        out = sharded(*concat_in, *concat_zeros)
    jax.block_until_ready(out)
    dt = (time.time() - t0) / iters
    return dt, out



# revision 2
# speedup vs baseline: 1.1861x; 1.1861x over previous
"""Trainium2 Bass kernel v2 for nn_Attention_Conv_surface (gnn_message_passing).

Math (per batch b):
  neighbors = vertices[idx]                          # (V, N, 3)
  dirn = normalize(neighbors - vertices[:, None])    # (V, N, 3)
  theta_d = sum_s max_n relu(dirn @ sdn_d)           # (V, K) for d in {q,k,v}
  qkv = theta @ W.T + b ; MHA over full VxV ; out = attn_out @ Wo.T + bo

v2 design (vs the hi/lo baseline):
  * Everything bf16 on the PE: scores stay in [-0.06, 0.11] and theta errors
    propagate to ~3e-3 final rel err (validated on host), 6x inside the 2e-2
    gate.  No hi/lo splits, no m-shift pass (exp never overflows).
  * theta: dirh tiles transposed to T4 [96, 512] (3 rows/neighbor); per-(chunk,
    neighbor) sparse lhsT [96,128] carries sdn at rows 3n.  One MM per neighbor
    yields t [128sk, 512v] fp32 in PSUM.  The max over n runs on a split
    consumer pipeline: NB neighbors go PSUM->ACT-copy(bf16,1024-wide)->DVE
    TT-max at 2x; the rest are DVE TT-max straight from PSUM at 1x.  relu is
    folded into the combine (scalar_tensor_tensor max/max with 0); the
    sum-over-s is a [I64;I64] fold matmul accumulating both chunks in PSUM.
  * Pair-split (cores 2b/2b+1 share batch b): each core computes k/v theta for
    its own 1024 vertices only; the halves are exchanged with an AllGather
    over replica pairs (attention is key-order invariant, so rank order works
    for both cores).  q-theta is computed while the collective flies.
  * attention: qh/kh projected via theta ones-row (bias folded, q pre-scaled
    by 1/4); v projected directly transposed ([128v,16] MMs).  Scores fp32 in
    wide PSUM, Exp on ACT at 1024 cols; PV with a leading ones column gives
    the softmax denominator in row 0; normalization via DVE reciprocal +
    ones17 broadcast matmul; per-head OT tiles keep every partition base at 0.
    Final projection accumulates 4 per-head MMs + a bias MM.

Sharding: 8 cores = (batch 0..3) x (vertex half 0..1); identical SPMD program,
half selected by feeding half-rolled per-core inputs (like the baseline).
"""

import numpy as np

BS, V, N, S, K, H = 4, 2048, 32, 4, 64, 4
DK = K // H
VQ = V // 2            # vertices/queries per core
NCH = 6                # sk chunks of 128 (768 = 3 dirs * 256)
NB = 20                # neighbors per unit on the ACT-copy path (rest DVE-direct)
EPS2 = 1e-24

_CACHE = {}


def _build_program():
    import concourse.bass as bass
    import concourse.mybir as mybir
    import concourse.tile as tile
    from concourse import bacc
    from contextlib import ExitStack

    f32 = mybir.dt.float32
    bf16 = mybir.dt.bfloat16
    Alu = mybir.AluOpType
    Act = mybir.ActivationFunctionType

    nc = bacc.Bacc("TRN2", target_bir_lowering=False, debug=False, num_devices=8)

    verts_d = nc.dram_tensor("verts", [VQ, 3], f32, kind="ExternalInput").ap()
    gath_d = nc.dram_tensor("gath", [VQ, N, 3], f32, kind="ExternalInput").ap()
    bank_d = nc.dram_tensor("bank", [96, NCH, N, 128], bf16, kind="ExternalInput").ap()
    identb_d = nc.dram_tensor("identb", [128, 128], bf16, kind="ExternalInput").ap()
    foldm_d = nc.dram_tensor("foldm", [128, K], bf16, kind="ExternalInput").ap()
    wq_d = nc.dram_tensor("wq", [65, H, DK], bf16, kind="ExternalInput").ap()
    wk_d = nc.dram_tensor("wk", [65, H, DK], bf16, kind="ExternalInput").ap()
    wv_d = nc.dram_tensor("wv", [65, H, DK], bf16, kind="ExternalInput").ap()
    wo_d = nc.dram_tensor("wo", [17, H, K], bf16, kind="ExternalInput").ap()
    bo_d = nc.dram_tensor("bo_row", [1, K], bf16, kind="ExternalInput").ap()
    out_d = nc.dram_tensor("out_t", [K, VQ], f32, kind="ExternalOutput").ap()

    with tile.TileContext(nc) as tc:
        with (
            tc.tile_pool(name="const", bufs=1) as cpool,
            tc.tile_pool(name="dram", bufs=1, space="DRAM") as dpool,
        ):
            # ---- persistent constants ----
            identb = cpool.tile([128, 128], bf16)
            nc.sync.dma_start(identb[:], identb_d[:])
            bank = cpool.tile([96, NCH, N, 128], bf16)
            foldm = cpool.tile([128, K], bf16)
            nc.sync.dma_start(foldm[:], foldm_d[:])
            wq = cpool.tile([65, H, DK], bf16)
            nc.sync.dma_start(wq[:], wq_d[:])
            wk = cpool.tile([65, H, DK], bf16)
            nc.sync.dma_start(wk[:], wk_d[:])
            wv = cpool.tile([65, H, DK], bf16)
            nc.sync.dma_start(wv[:], wv_d[:])
            wo = cpool.tile([17, H, K], bf16)
            nc.sync.dma_start(wo[:], wo_d[:])
            bo_row = cpool.tile([1, K], bf16)
            nc.sync.dma_start(bo_row[:], bo_d[:])
            ones17 = cpool.tile([1, 17], bf16)
            nc.vector.memset(ones17[:], 1.0)
            ones512 = cpool.tile([1, 512], bf16)
            nc.vector.memset(ones512[:], 1.0)

            th_q = cpool.tile([65, VQ], bf16)
            nc.vector.memset(th_q[64:65, :], 1.0)
            th_k = cpool.tile([65, V], bf16)
            nc.vector.memset(th_k[64:65, :], 1.0)
            th_v = cpool.tile([65, V], bf16)
            nc.vector.memset(th_v[64:65, :], 1.0)
            th_kl = cpool.tile([K, VQ], bf16)    # local k half (to exchange)
            th_vl = cpool.tile([K, VQ], bf16)
            t4s = [cpool.tile([96, 512], bf16, name=f"t4_{g}") for g in range(2)]

            cc_in = dpool.tile([2, K, VQ], bf16)
            cc_out = dpool.tile([2, 2, K, VQ], bf16)

            # ---- phase 1: edge math for own 1024 vertices ----
            ph1 = ExitStack()
            vtpool = ph1.enter_context(tc.tile_pool(name="vt", bufs=3))
            psB = ph1.enter_context(tc.tile_pool(name="psB", bufs=3, space="PSUM"))
            pst = ph1.enter_context(tc.tile_pool(name="pst", bufs=2, space="PSUM"))
            for vt in range(VQ // 128):
                vsl = slice(vt * 128, vt * 128 + 128)
                gath = vtpool.tile([128, N, 3], f32, tag="gath")
                nc.sync.dma_start(gath[:], gath_d[vsl, :, :])
                if vt == 1:
                    nc.sync.dma_start(bank[:, 2:6, :, :], bank_d[:, 2:6, :, :])
                cent = vtpool.tile([128, 3], f32, tag="cent")
                nc.sync.dma_start(cent[:], verts_d[vsl, :])
                diff = vtpool.tile([128, N, 3], f32, tag="diff")
                for c in range(3):
                    nc.vector.tensor_tensor(
                        out=diff[:, :, c],
                        in0=gath[:, :, c],
                        in1=cent[:, c:c + 1].to_broadcast([128, N]),
                        op=Alu.subtract,
                    )
                dsq = vtpool.tile([128, N, 3], f32, tag="dsq")
                nc.vector.tensor_tensor(out=dsq[:], in0=diff[:], in1=diff[:],
                                        op=Alu.mult)
                nsq = vtpool.tile([128, N], f32, tag="nsq")
                nc.vector.reduce_sum(nsq[:], dsq[:], axis=mybir.AxisListType.X)
                nc.vector.tensor_scalar_add(nsq[:], nsq[:], EPS2)
                rsq = vtpool.tile([128, N], f32, tag="rsq")
                nc.vector.reciprocal(rsq[:], nsq[:])
                inv = vtpool.tile([128, N], f32, tag="inv")
                nc.scalar.activation(inv[:], rsq[:], Act.Sqrt)
                dirh = vtpool.tile([128, N, 3], bf16, tag="dirh")
                nc.vector.tensor_tensor(
                    out=dirh[:],
                    in0=diff[:],
                    in1=inv[:].to_broadcast([128, N, 3]),
                    op=Alu.mult,
                )
                tp = psB.tile([96, 128], bf16, tag="wide")
                nc.tensor.transpose(
                    tp[:], dirh[:].rearrange("p a b -> p (a b)"), identb[:]
                )
                nc.vector.tensor_copy(
                    t4s[vt // 4][:, (vt % 4) * 128:(vt % 4) * 128 + 128], tp[:]
                )

            nc.sync.dma_start(bank[:, 0:2, :, :], bank_d[:, 0:2, :, :])

            # ---- phase 2: theta ----
            ph2 = ExitStack()
            ebpool = ph2.enter_context(tc.tile_pool(name="eb", bufs=3))
            acpool = ph2.enter_context(tc.tile_pool(name="ac", bufs=2))

            def unit(ch, g, fold_ps, start, stop):
                """One (chunk, vertex-group) theta unit: 32 MMs + max + fold."""
                accw = acpool.tile([128, 1024], bf16, tag="accw")
                acca = acpool.tile([128, 1024], f32, tag="acca")
                nwb = NB // 2              # wides on the ACT path
                nwa = 16 - nwb
                a_pos = set(range(nwb, 16))   # DVE-direct wides last: their DVE
                                              # tail overlaps the next unit's ACT head
                nb_seen = na_seen = 0
                for wi in range(16):
                    w = psB.tile([128, 1024], f32, tag="wide")
                    for j in range(2):
                        n = wi * 2 + j
                        nc.tensor.matmul(
                            out=w[:, j * 512:j * 512 + 512],
                            lhsT=bank[:, ch, n, :], rhs=t4s[g][:],
                            start=True, stop=True,
                        )
                    if wi not in a_pos:
                        eb = ebpool.tile([128, 1024], bf16, tag="eb")
                        nc.scalar.activation(eb[:], w[:], Act.Identity)
                        if nb_seen == 0:
                            nc.vector.tensor_copy(accw[:], eb[:])
                        else:
                            nc.vector.tensor_tensor(out=accw[:], in0=eb[:],
                                                    in1=accw[:], op=Alu.max)
                        nb_seen += 1
                    else:
                        if na_seen == 0:
                            nc.vector.tensor_copy(acca[:], w[:])
                        else:
                            nc.vector.tensor_tensor(out=acca[:], in0=w[:],
                                                    in1=acca[:], op=Alu.max)
                        na_seen += 1
                x1 = acpool.tile([128, 512], bf16, tag="x1")
                nc.vector.scalar_tensor_tensor(
                    out=x1[:], in0=acca[:, 0:512], scalar=0.0,
                    in1=acca[:, 512:1024], op0=Alu.max, op1=Alu.max,
                )
                x2 = acpool.tile([128, 512], bf16, tag="x2")
                nc.vector.tensor_tensor(out=x2[:], in0=accw[:, 0:512],
                                        in1=accw[:, 512:1024], op=Alu.max)
                r = acpool.tile([128, 512], bf16, tag="r")
                nc.vector.tensor_tensor(out=r[:], in0=x1[:], in1=x2[:],
                                        op=Alu.max)
                nc.tensor.matmul(out=fold_ps[:], lhsT=foldm[:], rhs=r[:],
                                 start=start, stop=stop)

            def run_dir(ch0, dst):
                for g in range(2):
                    gsl = slice(g * 512, g * 512 + 512)
                    fold_ps = pst.tile([K, 512], f32, tag="fold", bufs=2)
                    unit(ch0, g, fold_ps, True, False)
                    unit(ch0 + 1, g, fold_ps, False, True)
                    nc.scalar.activation(dst[0:K, gsl], fold_ps[:], Act.Identity)

            run_dir(2, th_kl)     # k local
            run_dir(4, th_vl)     # v local

            # exchange k/v halves across the pair
            nc.sync.dma_start(cc_in[0, :, :], th_kl[:])
            nc.sync.dma_start(cc_in[1, :, :], th_vl[:])
            nc.gpsimd.collective_compute(
                "AllGather", mybir.AluOpType.bypass,
                replica_groups=[[0, 1], [2, 3], [4, 5], [6, 7]],
                ins=[cc_in[:].opt()], outs=[cc_out[:].opt()],
            )

            run_dir(0, th_q)      # q local (overlaps collective)

            for rank in range(2):
                nc.sync.dma_start(th_k[0:K, rank * VQ:rank * VQ + VQ],
                                  cc_out[rank, 0, :, :])
                nc.sync.dma_start(th_v[0:K, rank * VQ:rank * VQ + VQ],
                                  cc_out[rank, 1, :, :])
            ph2.close()
            ph1.close()

            # ---- phase 3: attention ----
            at = ExitStack()
            sbp = at.enter_context(tc.tile_pool(name="sbp", bufs=2))
            e2p = at.enter_context(tc.tile_pool(name="e2p", bufs=4))
            psE = at.enter_context(tc.tile_pool(name="psE", bufs=2, space="PSUM"))
            psP = at.enter_context(tc.tile_pool(name="psP", bufs=2, space="PSUM"))

            otx = [cpool.tile([17, VQ], bf16, name=f"otx{h}") for h in range(H)]
            # zero-padded shapes keep PE MAC-occupancy high enough that the
            # HAM clock gate stays at full rate during attention
            va4 = cpool.tile([128, H, V // 128, 64], bf16)
            nc.vector.memset(va4[:], 0.0)
            nc.vector.memset(va4[:, :, :, 0:1], 1.0)
            qa4 = cpool.tile([96, H, VQ], bf16)
            nc.vector.memset(qa4[:], 0.0)
            ka4 = cpool.tile([96, H, V], bf16)
            nc.vector.memset(ka4[:], 0.0)
            pvs_all = cpool.tile([17, H, 2, 512], f32)

            # projections for all heads (overlap with q-theta units upstream)
            for h in range(H):
                for qs in range(2):
                    p = psP.tile([DK, 512], f32, tag="proj")
                    nc.tensor.matmul(out=p[:], lhsT=wq[:, h, :],
                                     rhs=th_q[:, qs * 512:qs * 512 + 512],
                                     start=True, stop=True)
                    nc.vector.tensor_copy(qa4[0:DK, h, qs * 512:qs * 512 + 512], p[:])
                for kt4 in range(4):
                    p = psP.tile([DK, 512], f32, tag="proj")
                    nc.tensor.matmul(out=p[:], lhsT=wk[:, h, :],
                                     rhs=th_k[:, kt4 * 512:kt4 * 512 + 512],
                                     start=True, stop=True)
                    nc.vector.tensor_copy(ka4[0:DK, h, kt4 * 512:kt4 * 512 + 512], p[:])
                for kt in range(V // 128):
                    p = psP.tile([128, DK], f32, tag="proj")
                    nc.tensor.matmul(out=p[:], lhsT=th_v[:, kt * 128:kt * 128 + 128],
                                     rhs=wv[:, h, :], start=True, stop=True)
                    nc.vector.tensor_copy(va4[:, h, kt, 1:17], p[:])

            # score/exp/PV streams: both query halves share each kt's
            # stationary (ka/va loaded once per kt), software-pipelined by two
            # stages so the in-order PE queue never waits on exp
            for h in range(H):
                pvs = [psP.tile([64, 512], f32, tag="pv", bufs=2, name=f"pv{h}_{i}")
                       for i in range(2)]
                es = []

                def pv_mms(kt):
                    for qs in range(2):
                        nc.tensor.matmul(
                            out=pvs[qs][:], lhsT=va4[:, h, kt, :],
                            rhs=es[kt][:, qs * 512:qs * 512 + 512],
                            start=(kt == 0), stop=(kt == 15),
                        )

                for kt in range(16):
                    w = psE.tile([128, 1024], f32, tag="wideE")
                    for qs in range(2):
                        nc.tensor.matmul(
                            out=w[:, qs * 512:qs * 512 + 512],
                            lhsT=ka4[:, h, kt * 128:kt * 128 + 128],
                            rhs=qa4[:, h, qs * 512:qs * 512 + 512],
                            start=True, stop=True,
                        )
                    e2 = e2p.tile([128, 1024], bf16, tag="e2")
                    nc.scalar.activation(e2[:], w[:], Act.Exp)
                    es.append(e2)
                    if kt >= 2:
                        pv_mms(kt - 2)
                pv_mms(14)
                pv_mms(15)
                for qs in range(2):
                    qsl = slice(qs * 512, qs * 512 + 512)
                    nc.vector.tensor_copy(pvs_all[:, h, qs, :], pvs[qs][0:17, :])
                    rz = sbp.tile([1, 512], bf16, tag="rz")
                    with nc.allow_low_precision(
                            reason="bf16 denom recip; 3e-3 overall validated"):
                        nc.vector.reciprocal(rz[:], pvs_all[0:1, h, qs, :])
                    bc = psP.tile([17, 512], f32, tag="proj")
                    nc.tensor.matmul(out=bc[:], lhsT=ones17[:], rhs=rz[:],
                                     start=True, stop=True)
                    nc.vector.tensor_tensor(out=otx[h][:, qsl], in0=bc[:],
                                            in1=pvs_all[:, h, qs, :], op=Alu.mult)

            # ---- final projection ----
            outsb = cpool.tile([K, VQ], f32)
            for qs in range(2):
                qsl = slice(qs * 512, qs * 512 + 512)
                f = psP.tile([K, 512], f32, tag="pv")
                nc.tensor.matmul(out=f[:], lhsT=bo_row[:], rhs=ones512[:],
                                 start=True, stop=False)
                for h in range(H):
                    nc.tensor.matmul(out=f[:], lhsT=wo[:, h, :],
                                     rhs=otx[h][:, qsl],
                                     start=False, stop=(h == H - 1))
                nc.vector.tensor_copy(outsb[:, qsl], f[:])
            nc.sync.dma_start(out_d[:], outsb[:])
            at.close()

    nc.compile()
    return nc


def _host_prep(inputs):
    """Build the 8 per-core input maps from full inputs."""
    import ml_dtypes

    bfd = ml_dtypes.bfloat16
    verts = np.ascontiguousarray(np.asarray(inputs["vertices"], dtype=np.float32))
    idx = np.ascontiguousarray(np.asarray(inputs["neighbor_index"]).astype(np.int64))

    sd = np.concatenate(
        [np.asarray(inputs["q_dirs"]), np.asarray(inputs["k_dirs"]),
         np.asarray(inputs["v_dirs"])], axis=1
    ).astype(np.float32)  # [3, 768]
    nrm = np.sqrt((sd * sd).sum(0, dtype=np.float32), dtype=np.float32)
    sdn = (sd / np.maximum(nrm, np.float32(1e-12))).astype(bfd)

    # sparse lhsT bank [96, NCH, N, 128]: rows 3n..3n+2 = sdn chunk
    bank = np.zeros((96, NCH, N, 128), bfd)
    for ch in range(NCH):
        blk = sdn[:, ch * 128:ch * 128 + 128]  # [3, 128]
        for n in range(N):
            bank[3 * n:3 * n + 3, ch, n, :] = blk

    foldm = np.zeros((128, K), bfd)
    foldm[0:K, :] = np.eye(K, dtype=np.float32)
    foldm[K:128, :] = np.eye(K, dtype=np.float32)

    def aug(Wkey, bkey, scale=1.0):
        W = np.asarray(inputs[Wkey], dtype=np.float32) * scale   # [K, K]
        b = np.asarray(inputs[bkey], dtype=np.float32) * scale   # [K]
        a = np.zeros((65, H, DK), bfd)
        for h in range(H):
            a[0:K, h, :] = W[h * DK:(h + 1) * DK, :].T
            a[K, h, :] = b[h * DK:(h + 1) * DK]
        return a

    wq = aug("Wq", "bq", 0.25)
    wk = aug("Wk", "bk")
    wv = aug("Wv", "bv")
    Wo = np.asarray(inputs["Wo"], dtype=np.float32)
    wo = np.zeros((17, H, K), bfd)
    for h in range(H):
        wo[1:17, h, :] = Wo[:, h * DK:(h + 1) * DK].T
    bo_row = np.asarray(inputs["bo"], dtype=np.float32).reshape(1, K).astype(bfd)

    common = {
        "bank": bank,
        "identb": np.eye(128, dtype=np.float32).astype(bfd),
        "foldm": foldm,
        "wq": wq, "wk": wk, "wv": wv, "wo": wo, "bo_row": bo_row,
    }

    in_maps = []
    for core in range(8):
        bb, half = core // 2, core % 2
        if half == 0:
            vb, ib = verts[bb], idx[bb]
        else:
            perm = np.concatenate([np.arange(VQ, V), np.arange(0, VQ)])
            vb = verts[bb][perm]
            ib = np.where(idx[bb][perm] >= VQ, idx[bb][perm] - VQ,
                          idx[bb][perm] + VQ)
        vb_own = np.ascontiguousarray(vb[0:VQ])
        ib_own = ib[0:VQ]
        in_maps.append({
            "verts": vb_own,
            "gath": np.ascontiguousarray(vb[ib_own]),
            **common,
        })
    return in_maps


def run(inputs, trace=False, trace_kwargs=None):
    from concourse.bass_utils import run_bass_kernel_spmd

    if "nc" not in _CACHE:
        _CACHE["nc"] = _build_program()
    nc = _CACHE["nc"]
    in_maps = _host_prep(inputs)
    res = run_bass_kernel_spmd(
        nc, in_maps, core_ids=list(range(8)), trace=trace,
        **(trace_kwargs or {}),
    )
    out = np.zeros((BS, V, K), np.float32)
    for core in range(8):
        bb, half = core // 2, core % 2
        ot = res.results[core]["out_t"]  # [64, 1024]
        out[bb, half * VQ:half * VQ + VQ, :] = ot.T
    return out, res


def kernel(**inputs) -> np.ndarray:
    out, _ = run(inputs, trace=False)
    return out
